# revision 1
# baseline (speedup 1.0000x reference)
"""AttentionMIL pooling kernel for 8 Trainium2 NeuronCores.

Math (per reference):
    h      = tanh(x @ W1 + b1)            [N, A]
    s      = h @ w2 + b2                  [N]
    a_r    = softmax over each bag of s   (segmented)
    pooled = sum_r a_r * x_r per bag      [B, D]
    out    = pooled @ Wh + bh             [B, 2]

Key transformations:
  * b2 cancels in the softmax exactly -> dropped.
  * scores are bounded (|s| <= ||w2||_1 ~ 9), so no per-bag max subtraction is
    needed: e_r = exp(s_r) stays comfortably in fp16/fp32 range.  The softmax
    becomes two plain segment sums, which combine across cores by addition.
  * Per 128-row tile: xe_r = e_r * x_r (one DVE tensor_scalar_mul), then
      U[b, :]  += Sel^T @ xe   and   den[b] += Sel^T @ e
    on the tensor engine, where Sel[r, b] = (seg_r == b) is a host-built fp16
    one-hot (free to build during sharding).  Both accumulate in PSUM across
    the whole core.
  * Host: U_tot = sum over cores, pooled = U/den, out = pooled @ Wh + bh.

Sharding: equal rows per core (65536).  Bags are global ids 0..63; per-core
partial (U, den) results combine by addition, so bag boundaries don't matter.

Data layout: host supplies x as fp16 twice: natural [R, D] (pooling matmul
moving operand, plain contiguous => dense 8KB DMA descriptors) and a
permuted-transposed copy (score matmul stationary operand), avoiding any
on-chip transpose.  fp16 (not bf16): x ~ N(0,1) and exp(s) <= ~20 are far
inside fp16 range, and the 10-bit mantissa keeps end-to-end relative error
~4e-4 (bf16 measures ~5e-3), while halving DMA traffic vs two f32 reads.

Row mapping: within an S*128-row super-block, SBUF partition p of subtile a
holds row (p*S + a) of the block (the natural layout of a contiguous DMA
into [128, S*256]).  sel and xt are host-permuted to match; the segment sums
are order-invariant so any per-core row permutation is legal.
"""

import numpy as np

import concourse.mybir as mybir
import concourse.tile as tile
from concourse import bacc
from concourse.bass_utils import run_bass_kernel_spmd

F16 = np.float16

N_CORES = 8
N_TOTAL = 524288
D = 256
A = 128
B = 64  # num bags
P = 128  # SBUF partitions
R = N_TOTAL // N_CORES  # rows per core
S = 32  # 128-row subtiles per super tile (DMA batch = 2 MiB)
G = 8  # subtiles per PSUM tanh group (one [128, G*A] tanh per group)
T = R // P  # 512 tiles per core
SUPERS = T // S

_NC_CACHE = {}


def build_nc(with_b1: bool, R=R, S=S, G=G, bufs=4, hbufs=2, n_cores=N_CORES, debug=False):
    T = R // P
    SUPERS = T // S
    assert S % G == 0
    dt = mybir.dt
    nc = bacc.Bacc("TRN2", target_bir_lowering=False, debug=debug, num_devices=n_cores)

    xnat_d = nc.dram_tensor("xnat", [R, D], dt.float16, kind="ExternalInput")
    xt_d = nc.dram_tensor("xt", [D, R], dt.float16, kind="ExternalInput")
    sel_d = nc.dram_tensor("sel", [P, T * B], dt.float16, kind="ExternalInput")
    w1_d = nc.dram_tensor("w1", [D, A], dt.float16, kind="ExternalInput")
    w2_d = nc.dram_tensor("w2", [P, G * A], dt.float16, kind="ExternalInput")
    if with_b1:
        b1_d = nc.dram_tensor("b1", [1, A], dt.float16, kind="ExternalInput")
    uout_d = nc.dram_tensor("uout", [B, D + 1], dt.float32, kind="ExternalOutput")

    # DRAM views matching the on-chip layouts.
    xn_view = xnat_d.ap().rearrange("(s p a) d -> s p (a d)", p=P, a=S)
    # xt columns are host-permuted so that within super s, column a*128+j of
    # the super's slab is the row held by partition j, subtile a.
    xt_view = xt_d.ap().rearrange("(h p) (s j) -> s p h j", p=P, s=SUPERS)
    sel_view = sel_d.ap().rearrange("p (s f) -> s p f", s=SUPERS)
    w1_view = w1_d.ap().rearrange("(h p) a -> p h a", p=P)

    with tile.TileContext(nc) as tc:
        with (
            tc.tile_pool(name="persist", bufs=1) as persist,
            tc.tile_pool(name="xn_pool", bufs=bufs) as xn_pool,
            tc.tile_pool(name="xt_pool", bufs=bufs) as xt_pool,
            tc.tile_pool(name="sel_pool", bufs=bufs) as sel_pool,
            tc.tile_pool(name="ht_pool", bufs=3) as ht_pool,
            tc.tile_pool(name="sc_pool", bufs=3) as sc_pool,
            tc.tile_pool(name="xe_pool", bufs=4) as xe_pool,
            tc.tile_pool(name="out_pool", bufs=1) as out_pool,
            tc.tile_pool(name="psum_u", bufs=1, space="PSUM") as psum_u,
            tc.tile_pool(name="psum_h", bufs=hbufs, space="PSUM") as psum_h,
        ):
            w1_sb = persist.tile([P, 2, A], dt.float16)
            nc.sync.dma_start(out=w1_sb, in_=w1_view)
            # w2 replicated G times along free dim (host-built) so one
            # tensor_tensor covers a whole tanh group.
            w2_sb = persist.tile([P, G * A], dt.float16)
            nc.sync.dma_start(out=w2_sb, in_=w2_d.ap())
            if with_b1:
                b1_sb = persist.tile([1, A], dt.float16)
                nc.sync.dma_start(out=b1_sb, in_=b1_d.ap())
                ones_sb = persist.tile([1, P], dt.float16)
                nc.vector.memset(ones_sb, 1.0)

            u_ps = psum_u.tile([B, D], dt.float32)
            den_ps = psum_u.tile([B, 1], dt.float32)

            def emit_eu(s, g, xn, sel_sb, e32, e16):
                """xe scaling + pooling/denominator matmuls for group g of
                super s (one group late so the PE never stalls on scores)."""
                for i in range(G):
                    a = g * G + i
                    t = s * S + a
                    xe = xe_pool.tile([P, D], dt.float16, name="xe")
                    nc.vector.tensor_scalar_mul(
                        out=xe,
                        in0=xn[:, a * D : (a + 1) * D],
                        scalar1=e32[:, i : i + 1],
                    )
                    nc.tensor.matmul(
                        u_ps,
                        lhsT=sel_sb[:, a * B : (a + 1) * B],
                        rhs=xe,
                        start=(t == 0),
                        stop=(t == T - 1),
                        skip_group_check=True,
                    )
                    nc.tensor.matmul(
                        den_ps,
                        lhsT=sel_sb[:, a * B : (a + 1) * B],
                        rhs=e16[:, i : i + 1],
                        start=(t == 0),
                        stop=(t == T - 1),
                        skip_group_check=True,
                    )

            prev = None
            for s in range(SUPERS):
                xn = xn_pool.tile([P, S * D], dt.float16, name="xn")
                nc.sync.dma_start(out=xn, in_=xn_view[s])
                xt_sb = xt_pool.tile([P, 2, S * P], dt.float16, name="xt_sb")
                nc.sync.dma_start(out=xt_sb, in_=xt_view[s])
                sel_sb = sel_pool.tile([P, S * B], dt.float16, name="sel_sb")
                nc.sync.dma_start(out=sel_sb, in_=sel_view[s])
                for g in range(S // G):
                    h_ps = psum_h.tile([P, G * A], dt.float32, name="h_ps")
                    for i in range(G):
                        a = g * G + i
                        nc.tensor.matmul(
                            h_ps[:, i * A : (i + 1) * A],
                            lhsT=xt_sb[:, 0, a * P : (a + 1) * P],
                            rhs=w1_sb[:, 0, :],
                            start=True,
                            stop=False,
                        )
                        nc.tensor.matmul(
                            h_ps[:, i * A : (i + 1) * A],
                            lhsT=xt_sb[:, 1, a * P : (a + 1) * P],
                            rhs=w1_sb[:, 1, :],
                            start=False,
                            stop=not with_b1,
                        )
                        if with_b1:
                            nc.tensor.matmul(
                                h_ps[:, i * A : (i + 1) * A],
                                lhsT=ones_sb,
                                rhs=b1_sb,
                                start=False,
                                stop=True,
                            )
                    h_t = ht_pool.tile([P, G * A], dt.float16, name="h_t")
                    nc.scalar.activation(
                        h_t, h_ps, mybir.ActivationFunctionType.Tanh
                    )
                    prod = ht_pool.tile([P, G * A], dt.float16, name="prod")
                    nc.vector.tensor_tensor(
                        out=prod, in0=h_t, in1=w2_sb, op=mybir.AluOpType.mult
                    )
                    # tree step: fold the two A/2 halves of each subtile with a
                    # 2x-mode fp16 add, then reduce half as many elements at 1x
                    prod3 = prod.rearrange("p (g a) -> p g a", g=G)
                    half = ht_pool.tile([P, G, A // 2], dt.float16, name="half")
                    nc.vector.tensor_tensor(
                        out=half,
                        in0=prod3[:, :, 0 : A // 2],
                        in1=prod3[:, :, A // 2 : A],
                        op=mybir.AluOpType.add,
                    )
                    quar = ht_pool.tile([P, G, A // 4], dt.float16, name="quar")
                    nc.vector.tensor_tensor(
                        out=quar,
                        in0=half[:, :, 0 : A // 4],
                        in1=half[:, :, A // 4 : A // 2],
                        op=mybir.AluOpType.add,
                    )
                    scores_g = sc_pool.tile([P, G], dt.float32, name="scores_g")
                    nc.vector.tensor_reduce(
                        out=scores_g,
                        in_=quar,
                        axis=mybir.AxisListType.X,
                        op=mybir.AluOpType.add,
                    )
                    e32 = sc_pool.tile([P, G], dt.float32, name="e32")
                    nc.scalar.activation(
                        e32, scores_g, mybir.ActivationFunctionType.Exp
                    )
                    e16 = sc_pool.tile([P, G], dt.float16, name="e16")
                    nc.vector.tensor_copy(e16, e32)
                    if prev is not None:
                        emit_eu(*prev)
                    prev = (s, g, xn, sel_sb, e32, e16)
            emit_eu(*prev)

            u_sb = out_pool.tile([B, D + 1], dt.float32)
            nc.vector.tensor_copy(u_sb[:, 0:D], u_ps)
            nc.vector.tensor_copy(u_sb[:, D : D + 1], den_ps)
            nc.sync.dma_start(out=uout_d.ap(), in_=u_sb)

    nc.compile()
    return nc


def _get_nc(with_b1: bool):
    key = ("v4", with_b1)
    if key not in _NC_CACHE:
        _NC_CACHE[key] = build_nc(with_b1)
    return _NC_CACHE[key]


def kernel(x, segment_ids, num_bags, W1, b1, w2, b2, Wh, bh):
    x = np.asarray(x)
    segment_ids = np.asarray(segment_ids)
    W1 = np.asarray(W1)
    b1 = np.asarray(b1)
    w2 = np.asarray(w2)
    Wh = np.asarray(Wh)
    bh = np.asarray(bh)
    num_bags = int(num_bags)
    assert x.shape == (N_TOTAL, D) and num_bags == B

    with_b1 = bool(np.any(b1))
    nc = _get_nc(with_b1)

    xb = x.astype(F16)
    w1_in = np.ascontiguousarray(W1.astype(F16))
    w2_in = np.ascontiguousarray(
        np.broadcast_to(np.tile(w2.astype(F16), G), (P, G * A))
    )

    in_maps = []
    for c in range(N_CORES):
        sl = slice(c * R, (c + 1) * R)
        xc = xb[sl]
        # xt: within super s, column s*S*P + a*P + j holds the row of
        # partition j, subtile a, i.e. original row s*S*P + j*S + a.
        xt = np.ascontiguousarray(
            xc.reshape(SUPERS, P, S, D).transpose(3, 0, 2, 1).reshape(D, R)
        )
        # sel[p, (s*S + a)*B + b] = 1 iff seg(row s*S*P + p*S + a) == b
        seg_c = (
            segment_ids[sl].reshape(SUPERS, P, S).transpose(1, 0, 2).reshape(P, T)
        )
        sel = np.zeros((P, T, B), dtype=F16)
        pp, tt = np.meshgrid(np.arange(P), np.arange(T), indexing="ij")
        sel[pp, tt, seg_c] = 1
        m = {
            "xnat": xc,  # contiguous slice, no copy needed
            "xt": xt,
            "sel": sel.reshape(P, T * B),
            "w1": w1_in,
            "w2": w2_in,
        }
        if with_b1:
            m["b1"] = np.ascontiguousarray(b1.astype(F16).reshape(1, A))
        in_maps.append(m)

    res = run_bass_kernel_spmd(nc, in_maps, core_ids=list(range(N_CORES)))

    U = np.zeros((B, D), np.float64)
    den = np.zeros((B,), np.float64)
    for c in range(N_CORES):
        u = res.results[c]["uout"].astype(np.float64)
        U += u[:, :D]
        den += u[:, D]
    pooled = np.where(den[:, None] > 0, U / np.where(den == 0, 1, den)[:, None], 0.0)
    out = pooled @ Wh.astype(np.float64) + bh.astype(np.float64)
    return out.astype(np.float32)

